# revision 1
# baseline (speedup 1.0000x reference)
"""Contextual loss (CX) kernel for Trainium2, 8 NeuronCores.

Sharding: data-parallel over (image, row-half): core c handles image c//2,
pred-rows [ (c%2)*2048, (c%2+1)*2048 ) of the 4096x4096 contextual matrix.
Each core computes, for its 2048 rows i:
    s_ij   = <phat_i, that_j>                (fp16 matmul, fp32 PSUM)
    smax_i = max_j s_ij
    b_i    = 1/(1 - smax_i + EPS)
    e_ij   = exp(b_i*(s_ij - smax_i))        (per-row-rescaled CX weights)
    rs_i   = sum_j e_ij
    M_j    = max(M_j, max_i e_ij / rs_i)     (column max of cx_ij)
and outputs M as a (128, 4096) tile (partition-dim partial maxes).
Host folds partitions + the two row-halves, means over j, -log, means over N.

This is mathematically identical to the reference: w_ij = exp((1-d~)/sigma)
differs from e_ij by a per-row constant factor which cancels in the row
normalization cx = w / rowsum(w).
"""

import os
import numpy as np
from contextlib import ExitStack

import concourse.bass as bass
import concourse.bacc as bacc
import concourse.mybir as mybir
import concourse.tile as tile
from concourse.bass_utils import run_bass_kernel_spmd

F32 = mybir.dt.float32
F16 = mybir.dt.float16
AX = mybir.AxisListType.X
ALU = mybir.AluOpType
ACTF = mybir.ActivationFunctionType

N_IMG, C, H, W = 4, 512, 64, 64
HW = H * W              # 4096
R = HW // 2             # 2048 rows per core
KB = C // 128           # 4 contraction blocks
NB = R // 128           # 16 row blocks per core
CH = 512                # column chunk (one PSUM bank)
NCH = HW // CH          # 8 chunks
NDVE = 3                # chunks copied out of PSUM by DVE (rowmax fused)
EPS = 1e-5
NEG_BIG = -1e30


def _build_nc():
    nc = bacc.Bacc("TRN2", target_bir_lowering=False, debug=False, num_devices=8)
    t_dram = nc.dram_tensor("t", [C, HW], F32, kind="ExternalInput").ap()
    p_dram = nc.dram_tensor("p", [C, R], F32, kind="ExternalInput").ap()
    m_dram = nc.dram_tensor("m_out", [128, HW], F16, kind="ExternalOutput").ap()

    with tile.TileContext(nc) as tc, ExitStack() as ctx:
        const = ctx.enter_context(tc.tile_pool(name="const", bufs=1))

        # persistent tiles
        ones = const.tile([128, 128], F16, tag="ones")
        nc.vector.memset(ones[:], 1.0)
        z512 = const.tile([128, CH], F32, tag="z512")
        nc.vector.memset(z512[:], 0.0)
        that = [const.tile([128, HW], F16, tag=f"that{k}", name=f"that{k}") for k in range(KB)]
        phat = [const.tile([128, R], F16, tag=f"phat{k}", name=f"phat{k}") for k in range(KB)]
        obs = [const.tile([1, 1], F32, tag=f"obs{i}", name=f"obs{i}")
               for i in range(NB)]

        # ---------------- preprocessing ----------------
        # sqp stays open for the whole kernel: its slots are read by the PE
        # (colsum matmuls) and must not be recycled into main-loop tiles,
        # which would add PE waits to ACT instructions (HW wait-slot limit).
        sqp = ctx.enter_context(tc.tile_pool(name="sqp", bufs=1))
        with (
            tc.tile_pool(name="raw", bufs=1) as raw,
            tc.tile_pool(name="prepps", bufs=1, space="PSUM") as prepps,
        ):
            traw = [raw.tile([128, HW], F32, tag=f"traw{k}", name=f"traw{k}") for k in range(KB)]
            praw = [raw.tile([128, R], F32, tag=f"praw{k}", name=f"praw{k}") for k in range(KB)]
            mu = [raw.tile([128, 1], F32, tag=f"mu{k}", name=f"mu{k}") for k in range(KB)]

            for k in range(KB):
                nc.sync.dma_start(traw[k][:], t_dram[k * 128:(k + 1) * 128, :])
            for k in range(KB):
                nc.sync.dma_start(praw[k][:], p_dram[k * 128:(k + 1) * 128, :])

            # target: mean over HW, center in place, square, channel-sum via PE
            cs_t = prepps.tile([128, HW], F32, tag="cs")
            for k in range(KB):
                sq = sqp.tile([128, HW], F16, tag="sq", bufs=2)
                nc.vector.reduce_sum(mu[k][:], traw[k][:], axis=AX)
                nc.vector.tensor_scalar(mu[k][:], mu[k][:], 1.0 / HW, None, ALU.mult)
                # center t in place; ACT square then has a single (DVE) dep
                nc.vector.tensor_scalar(
                    traw[k][:], traw[k][:], mu[k][:], None, ALU.subtract
                )
                nc.scalar.activation(sq[:], traw[k][:], ACTF.Square)
                for j in range(NCH):
                    nc.tensor.matmul(
                        cs_t[:, j * CH:(j + 1) * CH],
                        ones[:],
                        sq[:, j * CH:(j + 1) * CH],
                        start=(k == 0),
                        stop=(k == KB - 1),
                    )
            # invnorm = (colsum_sq)^-0.5 = exp(-0.5*ln(ss)); bounce through
            # SBUF to avoid in-place PSUM reads+writes on ACT
            invn = sqp.tile([128, HW], F32, tag="inv", name="invn")
            nc.scalar.activation(invn[:], cs_t[:], ACTF.Ln)
            nc.scalar.activation(cs_t[:], invn[:], ACTF.Exp, scale=-0.5)
            for k in range(KB):
                nc.vector.tensor_mul(that[k][:], traw[k][:], cs_t[:])

            # pred: same pipeline, centered with the *target* mean. The sq slots
            # are reused: their PE reads were already observed by the Ln above.
            cs_p = prepps.tile([128, R], F32, tag="cs")
            for k in range(KB):
                sq = sqp.tile([128, HW], F16, tag="sq", bufs=2, name="sqpp")
                nc.vector.tensor_scalar(
                    praw[k][:], praw[k][:], mu[k][:], None, ALU.subtract
                )
                # square on DVE: the reused sq slot's PE reads become a DVE
                # wait (DVE allows 2 waits; ACT allows only 1)
                nc.vector.tensor_mul(sq[:, :R], praw[k][:], praw[k][:])
                for j in range(R // CH):
                    nc.tensor.matmul(
                        cs_p[:, j * CH:(j + 1) * CH],
                        ones[:],
                        sq[:, j * CH:(j + 1) * CH],
                        start=(k == 0),
                        stop=(k == KB - 1),
                    )
            invn_p = sqp.tile([128, R], F32, tag="invp", name="invnp")
            nc.scalar.activation(invn_p[:], cs_p[:], ACTF.Ln)
            nc.scalar.activation(cs_p[:], invn_p[:], ACTF.Exp, scale=-0.5)
            for k in range(KB):
                nc.vector.tensor_mul(phat[k][:], praw[k][:], cs_p[:])

        # ---------------- main loop ----------------
        main = ctx.enter_context(tc.tile_pool(name="main", bufs=2))
        stats = ctx.enter_context(tc.tile_pool(name="stats", bufs=2))
        mainps = ctx.enter_context(tc.tile_pool(name="mainps", bufs=8, space="PSUM"))
        m_acc = main.tile([128, HW], F16, tag="m_acc", bufs=1)
        nc.vector.memset(m_acc[:], 0.0)
        reps = int(os.environ.get("CX_REPS", "1"))
        for ib in [i for _ in range(reps) for i in range(NB)]:
            s_t = main.tile([128, HW], F16, tag="s")
            e_t = main.tile([128, HW], F16, tag="e")
            scr_a = main.tile([128, CH], F16, tag="scr_a")
            scr_b = main.tile([128, CH], F16, tag="scr_b")
            cmax = stats.tile([128, 4], F32, tag="cmax")

            for jc in range(NCH):
                ps = mainps.tile([128, CH], F32, tag="ps")
                for kc in range(KB):
                    nc.tensor.matmul(
                        ps[:],
                        phat[kc][:, ib * 128:(ib + 1) * 128],
                        that[kc][:, jc * CH:(jc + 1) * CH],
                        start=(kc == 0),
                        stop=(kc == KB - 1),
                    )
                dst = s_t[:, jc * CH:(jc + 1) * CH]
                if jc < NDVE:
                    # fused PSUM->SBUF copy + row-max accumulation on DVE
                    nc.vector.tensor_scalar(
                        dst, ps[:], 1.0, None, ALU.mult, ALU.max,
                        accum_out=cmax[:, jc:jc + 1],
                    )
                else:
                    nc.scalar.copy(dst, ps[:])

            # rowmax of the ACT-copied chunks (fp16 tree at 2x, then 1x reduce)
            ck = lambda j: s_t[:, j * CH:(j + 1) * CH]
            nc.vector.tensor_max(scr_a[:], ck(3), ck(4))
            nc.vector.tensor_max(scr_b[:], ck(5), ck(6))
            nc.vector.tensor_max(scr_a[:], scr_a[:], ck(7))
            nc.vector.tensor_max(scr_a[:], scr_a[:], scr_b[:])
            nc.vector.reduce_max(cmax[:, 3:4], scr_a[:], axis=AX)

            smax = stats.tile([128, 1], F32, tag="smax")
            tmp = stats.tile([128, 1], F32, tag="tmp")
            b_t = stats.tile([128, 1], F32, tag="b")
            bias = stats.tile([128, 1], F32, tag="bias")
            rs = stats.tile([128, 1], F32, tag="rs")
            rinv = stats.tile([128, 1], F32, tag="rinv")

            nc.vector.reduce_max(smax[:], cmax[:], axis=AX)
            # b = 1/(1 + EPS - smax);  bias = -b*smax
            nc.vector.tensor_scalar(
                tmp[:], smax[:], -1.0, 1.0 + EPS, ALU.mult, ALU.add
            )
            nc.vector.reciprocal(b_t[:], tmp[:])
            nc.vector.scalar_tensor_tensor(
                bias[:], b_t[:], -1.0, smax[:], ALU.mult, ALU.mult
            )
            # 1-element ACT copy observes the newest DVE tick so the exp
            # needs only its single allowed (self) semaphore wait
            nc.scalar.copy(obs[ib][:], bias[0:1, :])
            nc.scalar.activation(
                e_t[:], s_t[:], ACTF.Exp, bias=bias[:], scale=b_t[:],
                accum_out=rs[:],
            )
            nc.vector.reciprocal(rinv[:], rs[:])
            # M = max(M, e * rinv)
            nc.vector.scalar_tensor_tensor(
                m_acc[:], e_t[:], rinv[:], m_acc[:], ALU.mult, ALU.max
            )

        nc.sync.dma_start(m_dram[:, :], m_acc[:])
    nc.compile()
    return nc


_NC_CACHE = {}


def _get_nc():
    if "nc" not in _NC_CACHE:
        _NC_CACHE["nc"] = _build_nc()
    return _NC_CACHE["nc"]


def kernel(pred, target, _trace=False):
    pred = np.asarray(pred, dtype=np.float32).reshape(N_IMG, C, HW)
    target = np.asarray(target, dtype=np.float32).reshape(N_IMG, C, HW)
    nc = _get_nc()
    in_maps = []
    for core in range(8):
        img, half = divmod(core, 2)
        in_maps.append({
            "t": np.ascontiguousarray(target[img]),
            "p": np.ascontiguousarray(pred[img, :, half * R:(half + 1) * R]),
        })
    res = run_bass_kernel_spmd(nc, in_maps, list(range(8)), trace=_trace)
    losses = []
    for img in range(N_IMG):
        m0 = res.results[2 * img]["m_out"].astype(np.float32).max(axis=0)
        m1 = res.results[2 * img + 1]["m_out"].astype(np.float32).max(axis=0)
        cx = np.maximum(m0, m1).mean()
        losses.append(-np.log(cx + EPS))
    out = np.float32(np.mean(losses))
    if _trace:
        return out, res
    return out



# revision 8
# speedup vs baseline: 1.1292x; 1.1292x over previous
"""Contextual loss (CX) kernel for Trainium2, 8 NeuronCores.

Sharding: data-parallel over (image, row-half): core c handles image c//2,
pred-rows [ (c%2)*2048, (c%2+1)*2048 ) of the 4096x4096 contextual matrix.

Math (per image, identical to the reference up to per-row constants that
cancel in the row-softmax):
    tc = t - mu,  pc = p - mu            (mu = target's mean feature)
    raw_ij  = <pc_i, tc_j>               (UN-normalized matmul, fp16)
    s~_ij   = raw_ij * invt_j            (column scale fused into PSUM evac)
    rmax_i  = max_j s~_ij                (fused into the same DVE op)
    smax_i  = rmax_i * invp_i            (row scale; invp per-partition via
                                          PE transpose of the colsum result)
    b_i     = 1/(1 - smax_i + EPS)
    e_ij    = exp( (b_i*invp_i) * s~_ij - b_i*smax_i )   (ACT, row-sum rs_i)
    M_j     = max(M_j, e_ij / rs_i)      (GpSimd, column-max partials)
Host folds partitions + row-halves, means over j, -log, means over N.
"""

import numpy as np
from contextlib import ExitStack

import concourse.bass as bass
import concourse.bacc as bacc
import concourse.mybir as mybir
import concourse.tile as tile
from concourse.bass_utils import run_bass_kernel_spmd

F32 = mybir.dt.float32
F16 = mybir.dt.float16
AX = mybir.AxisListType.X
ALU = mybir.AluOpType
ACTF = mybir.ActivationFunctionType

N_IMG, C, H, W = 4, 512, 64, 64
HW = H * W              # 4096
R = HW // 2             # 2048 rows per core
KB = C // 128           # 4 contraction blocks
NB = R // 128           # 16 row blocks per core
CH = 512                # column chunk (one PSUM bank)
NCH = HW // CH          # 8 chunks
EPS = 1e-5
NEG_BIG = -1e30


def _build_nc():
    nc = bacc.Bacc("TRN2", target_bir_lowering=False, debug=False, num_devices=8)
    t_dram = nc.dram_tensor("t", [C, HW], F32, kind="ExternalInput").ap()
    p_dram = nc.dram_tensor("p", [C, R], F32, kind="ExternalInput").ap()
    eye_dram = nc.dram_tensor("eye", [128, 128], F16, kind="ExternalInput").ap()
    m_dram = nc.dram_tensor("m_out", [128, HW], F16, kind="ExternalOutput").ap()

    with tile.TileContext(nc) as tc_ctx, ExitStack() as ctx:
        const = ctx.enter_context(tc_ctx.tile_pool(name="const", bufs=1))

        ones = const.tile([128, 128], F16, tag="ones")
        nc.vector.memset(ones[:], 1.0)
        eye = const.tile([128, 128], F16, tag="eye")
        nc.sync.dma_start(eye[:], eye_dram[:, :])

        # fp16 input tiles (centered in place during preproc)
        tct = [const.tile([128, HW], F16, tag=f"tct{k}", name=f"tct{k}") for k in range(KB)]
        pct = [const.tile([128, R], F16, tag=f"pct{k}", name=f"pct{k}") for k in range(KB)]
        invt = const.tile([128, HW], F16, tag="invt")        # column inv-norms (bcast)
        invp = const.tile([128, R], F16, tag="invp")         # row inv-norms (free layout)
        invp_t = const.tile([128, NB], F32, tag="invp_t")    # row inv-norms (partition layout)
        mu = [const.tile([128, 1], F32, tag=f"mu{k}", name=f"mu{k}") for k in range(KB)]

        # ---------------- input DMA (fp32 -> fp16 cast on SWDGE) ----------------
        for k in range(KB):
            nc.gpsimd.dma_start(tct[k][:], t_dram[k * 128:(k + 1) * 128, :])
        for k in range(KB):
            nc.gpsimd.dma_start(pct[k][:], p_dram[k * 128:(k + 1) * 128, :])

        # ---------------- preprocessing ----------------
        # sq tiles are read by the PE (colsum matmuls): keep the pool open for
        # the whole kernel so slots aren't recycled into main-loop tiles.
        sqp = ctx.enter_context(tc_ctx.tile_pool(name="sqp", bufs=1))
        scr = ctx.enter_context(tc_ctx.tile_pool(name="scr", bufs=1))
        with tc_ctx.tile_pool(name="prepps", bufs=1, space="PSUM") as prepps:
            # PSUM geometry: csa = banks 0-3 (t chunks 0-3), csb = banks 4-7
            # (t chunks 4-7, then reused for pred's colsums).
            cs_a = prepps.tile([128, HW // 2], F32, tag="csa")
            cs_b = prepps.tile([128, HW // 2], F32, tag="csb")

            musum = scr.tile([128, 1], F32, tag="musum", bufs=2)
            for k in range(KB):
                # mean of target features over HW (per channel)
                nc.vector.reduce_sum(musum[:], tct[k][:], axis=AX)
                nc.vector.tensor_scalar(mu[k][:], musum[:], 1.0 / HW, None, ALU.mult)
                # center t in place (fp16, 2x DVE mode)
                nc.vector.tensor_scalar(tct[k][:], tct[k][:], mu[k][:], None, ALU.subtract)
                # squares (ACT for k<3, DVE for the DMA-critical last block)
                sq = sqp.tile([128, HW], F16, tag="sq", bufs=2, name=f"sqt{k}")
                if k < KB - 1:
                    nc.scalar.activation(sq[:], tct[k][:], ACTF.Square)
                else:
                    nc.vector.tensor_mul(sq[:], tct[k][:], tct[k][:])
                for j in range(NCH // 2):
                    nc.tensor.matmul(
                        cs_a[:, j * CH:(j + 1) * CH], ones[:],
                        sq[:, j * CH:(j + 1) * CH],
                        start=(k == 0), stop=(k == KB - 1),
                    )
                for j in range(NCH // 2, NCH):
                    nc.tensor.matmul(
                        cs_b[:, (j - NCH // 2) * CH:(j - NCH // 2 + 1) * CH], ones[:],
                        sq[:, j * CH:(j + 1) * CH],
                        start=(k == 0), stop=(k == KB - 1),
                    )

            # invt = rsqrt(colsum) = exp(-0.5*ln(.)); bounce via SBUF scratch
            lnt = scr.tile([128, HW // 2], F32, tag="lnt", bufs=2)
            nc.scalar.activation(lnt[:], cs_a[:], ACTF.Ln)
            nc.scalar.activation(invt[:, :HW // 2], lnt[:], ACTF.Exp, scale=-0.5)
            lnt2 = scr.tile([128, HW // 2], F32, tag="lnt", bufs=2)
            nc.scalar.activation(lnt2[:], cs_b[:], ACTF.Ln)
            nc.scalar.activation(invt[:, HW // 2:], lnt2[:], ACTF.Exp, scale=-0.5)

            # pred: center with target's mu, squares, colsums into csb's banks
            cs_p = prepps.tile([128, R], F32, tag="csb")
            for k in range(KB):
                nc.vector.tensor_scalar(pct[k][:], pct[k][:], mu[k][:], None, ALU.subtract)
                sqk = sqp.tile([128, R], F16, tag="sqk", bufs=2, name=f"sqp{k}")
                nc.vector.tensor_mul(sqk[:], pct[k][:], pct[k][:])
                for j in range(R // CH):
                    nc.tensor.matmul(
                        cs_p[:, j * CH:(j + 1) * CH], ones[:],
                        sqk[:, j * CH:(j + 1) * CH],
                        start=(k == 0), stop=(k == KB - 1),
                    )
            lnp = scr.tile([128, R], F32, tag="lnp")
            nc.scalar.activation(lnp[:], cs_p[:], ACTF.Ln)
            nc.scalar.activation(invp[:], lnp[:], ACTF.Exp, scale=-0.5)

        # fold the column scale into t (tensor_tensor_reduce is broken on
        # this runtime, so the scale can't ride the PSUM evacuation).
        # Column-half-major order so the main loop's first chunks unblock
        # as early as possible.
        HH = HW // 4
        for jh in range(4):
            for k in range(KB):
                nc.vector.tensor_mul(
                    tct[k][:, jh * HH:(jh + 1) * HH],
                    tct[k][:, jh * HH:(jh + 1) * HH],
                    invt[:, jh * HH:(jh + 1) * HH],
                )

        # ---------------- main loop ----------------
        main = ctx.enter_context(tc_ctx.tile_pool(name="main", bufs=2))
        stats = ctx.enter_context(tc_ctx.tile_pool(name="stats", bufs=2))
        mainps = ctx.enter_context(tc_ctx.tile_pool(name="mainps", bufs=7, space="PSUM"))
        transps = ctx.enter_context(tc_ctx.tile_pool(name="transps", bufs=1, space="PSUM"))

        m_acc = main.tile([128, HW], F16, tag="m_acc", bufs=1)
        nc.vector.memset(m_acc[:], 0.0)

        # invp free-layout -> partition layout: PE transpose of each 128-wide
        # slice (all partitions equal), keep column 0.
        for ib in range(NB):
            tp = transps.tile([128, 128], F16, tag="tp")
            nc.tensor.transpose(tp[:], invp[:, ib * 128:(ib + 1) * 128], eye[:])
            nc.vector.tensor_copy(invp_t[:, ib:ib + 1], tp[:, 0:1])

        NACT = 4  # chunks evacuated by ACT (copy) + DVE row-max from SBUF
        for ib in range(NB):
            s_t = main.tile([128, HW], F16, tag="s")
            e_t = main.tile([128, HW], F16, tag="e")
            cmax = stats.tile([128, NCH], F32, tag="cmax")

            for jc in range(NCH):
                ps = mainps.tile([128, CH], F32, tag="ps")
                for kc in range(KB):
                    nc.tensor.matmul(
                        ps[:],
                        pct[kc][:, ib * 128:(ib + 1) * 128],
                        tct[kc][:, jc * CH:(jc + 1) * CH],
                        start=(kc == 0), stop=(kc == KB - 1),
                    )
                dst = s_t[:, jc * CH:(jc + 1) * CH]
                if jc < NCH - NACT:
                    # fused PSUM->SBUF copy + row-max accumulation on DVE
                    nc.vector.tensor_scalar(
                        dst, ps[:], 1.0, None, ALU.mult, ALU.max,
                        accum_out=cmax[:, jc:jc + 1],
                    )
                else:
                    # ACT evacuates PSUM; DVE reduces the max from SBUF fp16
                    nc.scalar.copy(dst, ps[:])
                    nc.vector.reduce_max(cmax[:, jc:jc + 1], dst, axis=AX)

            rawmax = stats.tile([128, 1], F32, tag="rawmax")
            smax = stats.tile([128, 1], F32, tag="smax")
            tmp = stats.tile([128, 1], F32, tag="tmp")
            b_t = stats.tile([128, 1], F32, tag="b")
            scale_e = stats.tile([128, 1], F32, tag="scale_e")
            bias_e = stats.tile([128, 1], F32, tag="bias_e")
            rs = stats.tile([128, 1], F32, tag="rs")
            rinv = stats.tile([128, 1], F32, tag="rinv")

            nc.vector.reduce_max(rawmax[:], cmax[:], axis=AX)
            nc.vector.tensor_mul(smax[:], rawmax[:], invp_t[:, ib:ib + 1])
            # b = 1/(1 + EPS - smax)
            nc.vector.tensor_scalar(tmp[:], smax[:], -1.0, 1.0 + EPS, ALU.mult, ALU.add)
            nc.vector.reciprocal(b_t[:], tmp[:])
            nc.vector.tensor_mul(scale_e[:], b_t[:], invp_t[:, ib:ib + 1])
            # bias = -b*smax
            nc.vector.scalar_tensor_tensor(
                bias_e[:], b_t[:], -1.0, smax[:], ALU.mult, ALU.mult
            )
            nc.scalar.activation(
                e_t[:], s_t[:], ACTF.Exp, bias=bias_e[:], scale=scale_e[:],
                accum_out=rs[:],
            )
            nc.vector.reciprocal(rinv[:], rs[:])
            # M = max(M, e * rinv)
            nc.vector.scalar_tensor_tensor(
                m_acc[:], e_t[:], rinv[:], m_acc[:], ALU.mult, ALU.max
            )

        nc.sync.dma_start(m_dram[:, :], m_acc[:])
    nc.compile()
    return nc


_NC_CACHE = {}


def _get_nc():
    if "nc" not in _NC_CACHE:
        _NC_CACHE["nc"] = _build_nc()
    return _NC_CACHE["nc"]


def kernel(pred, target, _trace=False):
    pred = np.asarray(pred, dtype=np.float32).reshape(N_IMG, C, HW)
    target = np.asarray(target, dtype=np.float32).reshape(N_IMG, C, HW)
    nc = _get_nc()
    eye = np.eye(128, dtype=np.float16)
    in_maps = []
    for core in range(8):
        img, half = divmod(core, 2)
        in_maps.append({
            "t": np.ascontiguousarray(target[img]),
            "p": np.ascontiguousarray(pred[img, :, half * R:(half + 1) * R]),
            "eye": eye,
        })
    res = run_bass_kernel_spmd(nc, in_maps, list(range(8)), trace=_trace)
    losses = []
    for img in range(N_IMG):
        m0 = res.results[2 * img]["m_out"].astype(np.float32).max(axis=0)
        m1 = res.results[2 * img + 1]["m_out"].astype(np.float32).max(axis=0)
        cx = np.maximum(m0, m1).mean()
        losses.append(-np.log(cx + EPS))
    out = np.float32(np.mean(losses))
    if _trace:
        return out, res
    return out


# revision 14
# speedup vs baseline: 1.2874x; 1.1401x over previous
"""Contextual loss (CX) kernel for Trainium2, 8 NeuronCores.

Sharding: data-parallel over (image, row-half): core c handles image c//2,
pred-rows [ (c%2)*2048, (c%2+1)*2048 ) of the 4096x4096 contextual matrix.

Math (per image, identical to the reference up to per-row constants that
cancel in the row-softmax):
    tc = t - mu,  pc = p - mu              (mu = target's mean feature)
    tc' = tc * invt_h                      (invt_h = Dsqrt(||tc_j||^2) = 1/(2||tc_j||))
    s~_ij  = <pc_i, tc'_j>                 (fp16 matmul; = cos_ij * ||pc_i|| / 2)
    rmax_i = max_j s~_ij                   (fused into the PSUM evacuation)
    smax_i = 4 * rmax_i * invp_h_i         (invp_h per-partition via PE transpose)
    b_i    = 1/(1 - smax_i + EPS)
    e_ij   = exp( (4*b_i*invp_h_i) * s~_ij - b_i*smax_i ),  rs_i = sum_j e_ij
    M_j    = max(M_j, e_ij / rs_i)         (ACT mul + DVE TT-max ping-pong)
Host folds partitions + row-halves, means over j, -log, means over N.
"""

import numpy as np
from contextlib import ExitStack

import concourse.bass as bass
import concourse.bacc as bacc
import concourse.mybir as mybir
import concourse.tile as tile
from concourse.bass_utils import run_bass_kernel_spmd

F32 = mybir.dt.float32
F16 = mybir.dt.float16
AX = mybir.AxisListType.X
ALU = mybir.AluOpType
ACTF = mybir.ActivationFunctionType

N_IMG, C, H, W = 4, 512, 64, 64
HW = H * W              # 4096
R = HW // 2             # 2048 rows per core
KB = C // 128           # 4 contraction blocks
NB = R // 128           # 16 row blocks per core
CH = 512                # matmul free-dim chunk (one PSUM bank)
CC = 1024               # evacuation chunk (two PSUM banks)
NCC = HW // CC          # 4 evac chunks per row block
EPS = 1e-5


def _build_nc():
    nc = bacc.Bacc("TRN2", target_bir_lowering=False, debug=False, num_devices=8)
    t_dram = nc.dram_tensor("t", [C, HW], F32, kind="ExternalInput").ap()
    p_dram = nc.dram_tensor("p", [C, R], F32, kind="ExternalInput").ap()
    eye_dram = nc.dram_tensor("eye", [128, 128], F16, kind="ExternalInput").ap()
    m_dram = nc.dram_tensor("m_out", [128, HW], F16, kind="ExternalOutput").ap()

    with tile.TileContext(nc) as tc_ctx, ExitStack() as ctx:
        const = ctx.enter_context(tc_ctx.tile_pool(name="const", bufs=1))

        ones = const.tile([128, 128], F16, tag="ones")
        nc.vector.memset(ones[:], 1.0)
        eye = const.tile([128, 128], F16, tag="eye")
        nc.sync.dma_start(eye[:], eye_dram[:, :])

        tct = [const.tile([128, HW], F16, tag=f"tct{k}", name=f"tct{k}") for k in range(KB)]
        pct = [const.tile([128, R], F16, tag=f"pct{k}", name=f"pct{k}") for k in range(KB)]
        invt = const.tile([128, HW], F16, tag="invt")        # 1/(2*colnorm) bcast
        invp = const.tile([128, R], F16, tag="invp")         # 1/(2*rownorm) free layout
        invp_t = const.tile([128, NB], F32, tag="invp_t")    # same, partition layout
        mu = [const.tile([128, 1], F32, tag=f"mu{k}", name=f"mu{k}") for k in range(KB)]

        # ---------------- input DMA (fp32 -> fp16 cast on SWDGE) ----------------
        for k in range(KB):
            nc.gpsimd.dma_start(tct[k][:], t_dram[k * 128:(k + 1) * 128, :])
        for k in range(KB):
            nc.gpsimd.dma_start(pct[k][:], p_dram[k * 128:(k + 1) * 128, :])

        # ---------------- preprocessing ----------------
        sqp = ctx.enter_context(tc_ctx.tile_pool(name="sqp", bufs=1))
        with tc_ctx.tile_pool(name="prepps", bufs=1, space="PSUM") as prepps:
            # PSUM geometry: csa = banks 0-3 (t chunks 0-3), csb = banks 4-7
            # (t chunks 4-7, then reused for pred's colsums).
            cs_a = prepps.tile([128, HW // 2], F32, tag="csa")
            cs_b = prepps.tile([128, HW // 2], F32, tag="csb")

            musum = sqp.tile([128, 1], F32, tag="musum", bufs=2)
            for k in range(KB):
                nc.vector.reduce_sum(musum[:], tct[k][:], axis=AX)
                # store NEGATIVE mean: works as both DVE add-operand and ACT bias
                nc.vector.tensor_scalar(mu[k][:], musum[:], -1.0 / HW, None, ALU.mult)
                # center t in place (fp16, 2x DVE mode)
                nc.vector.tensor_scalar(tct[k][:], tct[k][:], mu[k][:], None, ALU.add)
                sq = sqp.tile([128, HW], F16, tag="sq", bufs=2, name=f"sqt{k}")
                if k < KB - 1:
                    nc.scalar.activation(sq[:], tct[k][:], ACTF.Square)
                else:
                    # last block on DVE: its square gates the whole invt chain
                    nc.vector.tensor_mul(sq[:], tct[k][:], tct[k][:])
                for j in range(4):
                    nc.tensor.matmul(
                        cs_a[:, j * CH:(j + 1) * CH], ones[:],
                        sq[:, j * CH:(j + 1) * CH],
                        start=(k == 0), stop=(k == KB - 1),
                    )
                for j in range(4, 8):
                    nc.tensor.matmul(
                        cs_b[:, (j - 4) * CH:(j - 3) * CH], ones[:],
                        sq[:, j * CH:(j + 1) * CH],
                        start=(k == 0), stop=(k == KB - 1),
                    )

            # invt = rsqrt(colsum) = exp(-0.5*ln(.)) (Rsqrt/Dsqrt unavailable)
            lnt = sqp.tile([128, HW // 2], F32, tag="lnt", bufs=2)
            nc.scalar.activation(lnt[:], cs_a[:], ACTF.Ln)
            nc.scalar.activation(invt[:, :HW // 2], lnt[:], ACTF.Exp, scale=-0.5)
            lnt2 = sqp.tile([128, HW // 2], F32, tag="lnt", bufs=2)
            nc.scalar.activation(lnt2[:], cs_b[:], ACTF.Ln)
            nc.scalar.activation(invt[:, HW // 2:], lnt2[:], ACTF.Exp, scale=-0.5)

            # pred: center with target's mu (ACT, keeps DVE free), squares on
            # GpSimd (idle) except the chain-critical last block on DVE.
            cs_p = prepps.tile([128, R], F32, tag="csb")
            for k in range(KB):
                nc.scalar.activation(
                    pct[k][:], pct[k][:], ACTF.Identity, bias=mu[k][:], scale=1.0
                )
                sqk = sqp.tile([128, R], F16, tag="sqk", bufs=2, name=f"sqp{k}")
                if k < KB - 1:
                    nc.gpsimd.tensor_tensor(sqk[:], pct[k][:], pct[k][:], ALU.mult)
                else:
                    nc.vector.tensor_mul(sqk[:], pct[k][:], pct[k][:])
                for j in range(R // CH):
                    nc.tensor.matmul(
                        cs_p[:, j * CH:(j + 1) * CH], ones[:],
                        sqk[:, j * CH:(j + 1) * CH],
                        start=(k == 0), stop=(k == KB - 1),
                    )
            lnp = sqp.tile([128, R], F32, tag="lnp")
            nc.scalar.activation(lnp[:], cs_p[:], ACTF.Ln)
            nc.scalar.activation(invp[:], lnp[:], ACTF.Exp, scale=-0.5)

        # fold the column scale into t (column-half-major so the main loop's
        # first chunks unblock earliest)
        for jh in range(4):
            for k in range(KB):
                nc.vector.tensor_mul(
                    tct[k][:, jh * CC:(jh + 1) * CC],
                    tct[k][:, jh * CC:(jh + 1) * CC],
                    invt[:, jh * CC:(jh + 1) * CC],
                )

        # ---------------- main loop ----------------
        main = ctx.enter_context(tc_ctx.tile_pool(name="main", bufs=2))
        stats = ctx.enter_context(tc_ctx.tile_pool(name="stats", bufs=2))
        mainps = ctx.enter_context(tc_ctx.tile_pool(name="mainps", bufs=3, space="PSUM"))
        transps = ctx.enter_context(tc_ctx.tile_pool(name="transps", bufs=1, space="PSUM"))

        # invp free-layout -> partition layout: PE transpose of each 128-wide
        # slice (all partitions equal), keep column 0.
        for ib in range(NB):
            tp = transps.tile([128, 128], F16, tag="tp")
            nc.tensor.transpose(tp[:], invp[:, ib * 128:(ib + 1) * 128], eye[:])
            nc.vector.tensor_copy(invp_t[:, ib:ib + 1], tp[:, 0:1])

        m_prev = main.tile([128, HW], F16, tag="m")
        nc.vector.memset(m_prev[:], 0.0)

        e_tiles = [None] * NB
        rinv_tiles = [None] * NB

        def finalize(ib):
            """e' = e*rinv on ACT, then column-max fold on DVE (ping-pong)."""
            nonlocal m_prev
            ep = main.tile([128, HW], F16, tag="ep")
            nc.scalar.mul(ep[:], e_tiles[ib][:], rinv_tiles[ib][:])
            m_cur = main.tile([128, HW], F16, tag="m")
            nc.vector.tensor_tensor(m_cur[:], ep[:], m_prev[:], ALU.max)
            m_prev = m_cur

        for ib in range(NB):
            s_t = main.tile([128, HW], F16, tag="s")
            e_t = main.tile([128, HW], F16, tag="e")
            cmax = stats.tile([128, NCC], F32, tag="cmax")

            for c in range(NCC):
                ps = mainps.tile([128, CC], F32, tag="ps")
                for half in range(2):
                    for kc in range(KB):
                        nc.tensor.matmul(
                            ps[:, half * CH:(half + 1) * CH],
                            pct[kc][:, ib * 128:(ib + 1) * 128],
                            tct[kc][:, (2 * c + half) * CH:(2 * c + half + 1) * CH],
                            start=(kc == 0), stop=(kc == KB - 1),
                        )
                # fused PSUM->SBUF copy + row-max accumulation on DVE
                nc.vector.tensor_scalar(
                    s_t[:, c * CC:(c + 1) * CC], ps[:], 1.0, None, ALU.mult, ALU.max,
                    accum_out=cmax[:, c:c + 1],
                )

            rawmax = stats.tile([128, 1], F32, tag="rawmax")
            smax = stats.tile([128, 1], F32, tag="smax")
            tmp = stats.tile([128, 1], F32, tag="tmp")
            b_t = stats.tile([128, 1], F32, tag="b")
            scale_e = stats.tile([128, 1], F32, tag="scale_e")
            bias_e = stats.tile([128, 1], F32, tag="bias_e")
            rs = stats.tile([128, 1], F32, tag="rs")
            rinv = stats.tile([128, 1], F32, tag="rinv")

            nc.vector.reduce_max(rawmax[:], cmax[:], axis=AX)
            # smax = rawmax * invp
            nc.vector.tensor_mul(smax[:], rawmax[:], invp_t[:, ib:ib + 1])
            # b = 1/(1 + EPS - smax)
            nc.vector.tensor_scalar(tmp[:], smax[:], -1.0, 1.0 + EPS, ALU.mult, ALU.add)
            nc.vector.reciprocal(b_t[:], tmp[:])
            nc.vector.tensor_mul(scale_e[:], b_t[:], invp_t[:, ib:ib + 1])
            nc.vector.scalar_tensor_tensor(
                bias_e[:], b_t[:], -1.0, smax[:], ALU.mult, ALU.mult
            )
            nc.scalar.activation(
                e_t[:], s_t[:], ACTF.Exp, bias=bias_e[:], scale=scale_e[:],
                accum_out=rs[:],
            )
            nc.vector.reciprocal(rinv[:], rs[:])
            e_tiles[ib] = e_t
            rinv_tiles[ib] = rinv
            # one-iteration-delayed normalization keeps the ACT FIFO flowing
            if ib > 0:
                finalize(ib - 1)
        finalize(NB - 1)

        nc.sync.dma_start(m_dram[:, :], m_prev[:])
    nc.compile()
    return nc


_NC_CACHE = {}


def _get_nc():
    if "nc" not in _NC_CACHE:
        _NC_CACHE["nc"] = _build_nc()
    return _NC_CACHE["nc"]


def kernel(pred, target, _trace=False):
    pred = np.asarray(pred, dtype=np.float32).reshape(N_IMG, C, HW)
    target = np.asarray(target, dtype=np.float32).reshape(N_IMG, C, HW)
    nc = _get_nc()
    eye = np.eye(128, dtype=np.float16)
    in_maps = []
    for core in range(8):
        img, half = divmod(core, 2)
        in_maps.append({
            "t": np.ascontiguousarray(target[img]),
            "p": np.ascontiguousarray(pred[img, :, half * R:(half + 1) * R]),
            "eye": eye,
        })
    res = run_bass_kernel_spmd(nc, in_maps, list(range(8)), trace=_trace)
    losses = []
    for img in range(N_IMG):
        m0 = res.results[2 * img]["m_out"].astype(np.float32).max(axis=0)
        m1 = res.results[2 * img + 1]["m_out"].astype(np.float32).max(axis=0)
        cx = np.maximum(m0, m1).mean()
        losses.append(-np.log(cx + EPS))
    out = np.float32(np.mean(losses))
    if _trace:
        return out, res
    return out
